# revision 10
# baseline (speedup 1.0000x reference)
"""Trainium2 Bass kernel for gated short-time-warp + Conv1d (nn_GW_Conv1D).

Reference computation (per batch element b, C=64 channels, T=32768):
  g = tanh(einsum('ct,c->t', x, est_w)) * 0.5            # velocity, |g| <= 0.5
  d = flow(g)    per 256-window (scaling & squaring, 4 iters), |d| <= 0.5
  xw = interp1d(x, p + d)   per window                    # forward warp
  y = conv1d(xw, conv_w, conv_b, k=3, SAME)               # channel mixing
  d_inv = flow(-g); out = interp1d(y, p + d_inv)          # inverse warp

Because |d| < 1 always, every interpolation touches only nearest neighbours,
so the warps are 3-term elementwise expressions with relu-split coefficients:
  out = x*am + x[-1]*dn + x[+1]*dp,  dn=relu(-d), dp=relu(d), am=1-dn-dp
with dn/dp zeroed at window edges (replicates jnp.clip at the borders).

Sharding: pure data parallelism, batch b -> core b (8 cores).

Layout: everything stays in "conv layout" (128 partitions = channel + 64*half,
16384 time columns). The warp coefficients are channel-invariant, so they are
computed compactly in window layout (128 windows x 256), flattened to one row
per (half, chunk), and broadcast across the 64 channel partitions per half
with large-descriptor SBUF->SBUF DMAs. This avoids the per-channel 512B
scatter/gather layout conversions entirely.

g is produced by x-stationary matmuls (output = time-on-partitions in PSUM),
moved to window layout via two XBAR DMA transposes + a tiny row permute.

The time axis is processed in 8 chunks of 2048 columns, fully pipelined:
broadcast -> fwd warp (DVE+Pool) -> conv (PE, fp16) -> inv warp -> store.
Chunk 7 is warped first so the cross-half conv halo columns are ready before
conv of chunk 0 runs.
"""
import sys

sys.path.insert(0, "/opt/trn_rl_repo")

import numpy as np
from contextlib import ExitStack

import concourse.bass as bass
import concourse.tile as tile
from concourse import bacc, mybir
from concourse.bass_interp import get_hw_module
from concourse import bass_utils

F32 = mybir.dt.float32
F16 = mybir.dt.float16
AF = mybir.ActivationFunctionType
ALU = mybir.AluOpType

NCORES = 8
C, T, W = 64, 32768, 256
H = T // 2            # 16384 columns per half (stacked-halves conv layout)
CH = 1024             # pipeline chunk width
NCH = H // CH         # 16 chunks
SUB = 512             # conv sub-chunk (one PSUM bank)
FLOW_ITERS = 4


def _flow_iteration(nc, pool, d2):
    """One scaling-and-squaring step on d2 (128, 512) fp16 = [d_fwd | d_inv].
    d2 <- d2 + interp1d(d2, p + d2), per 256-column window."""
    dn = pool.tile([128, 512], F16, tag="fl_dn")
    dp = pool.tile([128, 512], F16, tag="fl_dp")
    nc.scalar.activation(dn[:], d2[:], AF.Relu, scale=-1.0)
    nc.scalar.activation(dp[:], d2[:], AF.Relu)
    # window-edge masking (jnp.clip at borders)
    nc.gpsimd.memset(dn[:, 0:1], 0.0)
    nc.gpsimd.memset(dn[:, 256:257], 0.0)
    nc.gpsimd.memset(dp[:, 255:256], 0.0)
    nc.gpsimd.memset(dp[:, 511:512], 0.0)
    am = pool.tile([128, 512], F16, tag="fl_am")
    nc.vector.tensor_tensor(am[:], dn[:], dp[:], ALU.add)
    nc.vector.tensor_scalar(am[:], am[:], -1.0, 1.0, ALU.mult, ALU.add)
    itp = pool.tile([128, 512], F16, tag="fl_itp")
    tmp = pool.tile([128, 512], F16, tag="fl_tmp")
    nc.vector.tensor_tensor(itp[:], d2[:], am[:], ALU.mult)
    # left-neighbour term (dn masked at window starts -> cross-window leak *0)
    nc.vector.tensor_tensor(tmp[:, 1:512], d2[:, 0:511], dn[:, 1:512], ALU.mult)
    nc.vector.tensor_tensor(itp[:, 1:512], itp[:, 1:512], tmp[:, 1:512], ALU.add)
    # right-neighbour term
    nc.vector.tensor_tensor(tmp[:, 0:511], d2[:, 1:512], dp[:, 0:511], ALU.mult)
    nc.vector.tensor_tensor(itp[:, 0:511], itp[:, 0:511], tmp[:, 0:511], ALU.add)
    nc.vector.tensor_tensor(d2[:], d2[:], itp[:], ALU.add)


def _build_module():
    nc = bacc.Bacc("TRN2", target_bir_lowering=False, debug=False,
                   enable_asserts=False, num_devices=NCORES)
    x = nc.dram_tensor("x", (C, T), F32, kind="ExternalInput").ap()
    ew = nc.dram_tensor("ew", (128, 2), F16, kind="ExternalInput").ap()
    cw = nc.dram_tensor("cw", (128, 384), F16, kind="ExternalInput").ap()
    cb = nc.dram_tensor("cb", (128, 1), F32, kind="ExternalInput").ap()
    y = nc.dram_tensor("y", (C, T), F32, kind="ExternalOutput").ap()

    x_hc = x.rearrange("c (h t) -> h c t", h=2)          # (2, 64, H)
    y_hc = y.rearrange("c (h t) -> h c t", h=2)

    with tile.TileContext(nc) as tc, ExitStack() as ctx:
        sm = ctx.enter_context(tc.tile_pool(name="sm", bufs=1))
        big = ctx.enter_context(tc.tile_pool(name="big", bufs=1))
        f32s = ctx.enter_context(tc.tile_pool(name="f32s", bufs=2))
        cfF = ctx.enter_context(tc.tile_pool(name="cfF", bufs=2))
        cfI = ctx.enter_context(tc.tile_pool(name="cfI", bufs=2))
        wrk = ctx.enter_context(tc.tile_pool(name="wrk", bufs=2))
        psE = ctx.enter_context(tc.tile_pool(name="psE", bufs=1, space="PSUM"))
        psC = ctx.enter_context(tc.tile_pool(name="psC", bufs=4, space="PSUM"))

        ew_sb = sm.tile([128, 2], F16, tag="ew")
        nc.sync.dma_start(ew_sb[:], ew)
        cw_sb = sm.tile([128, 384], F16, tag="cw")
        nc.sync.dma_start(cw_sb[:], cw)
        cb_sb = sm.tile([128, 1], F32, tag="cb")
        nc.sync.dma_start(cb_sb[:], cb)

        x16 = big.tile([128, H], F16)          # signal, conv layout
        xw = big.tile([128, H + 2], F16)       # warped, col0/col H+1 = halo
        yc = big.tile([128, H], F16)           # conv output
        nc.gpsimd.memset(xw[0:64, 0:1], 0.0)          # t=-1 of half0
        nc.gpsimd.memset(xw[64:128, H + 1:H + 2], 0.0)  # t=H of half1

        # ---------------- Stage A: load + cast + einsum g -------------------
        # einsum via x-stationary matmuls: out partition = time-within-128,
        # psum col = 2*subchunk + half.
        # psum column for (subchunk jj, half h) is chosen so that XBAR
        # transposes of the two column halves land DIRECTLY in window layout:
        # col(jj, h) = (jj>>1) + 128*(jj&1) + 64*h.
        g_ps = psE.tile([128, 256], F32, tag="gps")
        g_v = g_ps.rearrange("q (a b) -> q a b", b=64)
        for k in range(NCH):
            a = k * CH
            xf = f32s.tile([128, CH], F32, tag="f32s")
            nc.sync.dma_start(xf[:], x_hc[:, :, a:a + CH])
            nc.scalar.copy(x16[:, a:a + CH], xf[:])
            for j in range(CH // 128):
                jj = (CH // 128) * k + j
                a0, b0 = 2 * (jj & 1), jj >> 1
                nc.tensor.matmul(g_v[:, a0:a0 + 2, b0:b0 + 1],
                                 x16[:, 128 * jj:128 * jj + 128], ew_sb[:],
                                 start=True, stop=True)

        # ---------------- Stage B: g -> window layout -----------------------
        g16 = sm.tile([128, 256], F16, tag="g16")
        nc.scalar.copy(g16[:], g_ps[:])
        # gw16[f=64h+l, w] = g(h, 256l + w): two direct XBAR transposes
        gw16 = sm.tile([128, 256], F16, tag="gw16")
        nc.sync.dma_start_transpose(gw16[:, 0:128], g16[:, 0:128])
        nc.sync.dma_start_transpose(gw16[:, 128:256], g16[:, 128:256])

        # ---------------- Stage C: flow + coefficients ----------------------
        gth = sm.tile([128, 256], F16, tag="gth")
        nc.scalar.activation(gth[:], gw16[:], AF.Tanh)
        d2 = sm.tile([128, 512], F16, tag="d2")           # [d_fwd | d_inv]
        nc.vector.tensor_scalar_mul(d2[:, 0:256], gth[:], 0.5 / 16.0)
        nc.vector.tensor_scalar_mul(d2[:, 256:512], gth[:], -0.5 / 16.0)
        for _ in range(FLOW_ITERS):
            _flow_iteration(nc, sm, d2)
        dn2 = sm.tile([128, 512], F16, tag="cf_dn")
        dp2 = sm.tile([128, 512], F16, tag="cf_dp")
        nc.scalar.activation(dn2[:], d2[:], AF.Relu, scale=-1.0)
        nc.scalar.activation(dp2[:], d2[:], AF.Relu)
        nc.gpsimd.memset(dn2[:, 0:1], 0.0)
        nc.gpsimd.memset(dn2[:, 256:257], 0.0)
        nc.gpsimd.memset(dp2[:, 255:256], 0.0)
        nc.gpsimd.memset(dp2[:, 511:512], 0.0)
        am2 = sm.tile([128, 512], F16, tag="cf_am")
        nc.vector.tensor_tensor(am2[:], dn2[:], dp2[:], ALU.add)
        nc.vector.tensor_scalar(am2[:], am2[:], -1.0, 1.0, ALU.mult, ALU.add)

        # ---------------- Stage D: flatten coefficients ---------------------
        # flat[k, CH*h + j] = coef(half h, t = CH*k + j); one (NCH, 2*CH) tile
        # per coefficient, written by 2 DMAs (64 x 512B descriptors each).
        flats = {}
        for nm, srcv, off in (("amf", am2, 0), ("dnf", dn2, 0), ("dpf", dp2, 0),
                              ("ami", am2, 256), ("dni", dn2, 256),
                              ("dpi", dp2, 256)):
            fl = sm.tile([NCH, 2 * CH], F16, tag="fl_" + nm)
            for h in (0, 1):
                nc.sync.dma_start(fl[0:NCH, CH * h:CH * h + CH],
                                  srcv[64 * h:64 * h + 64, off:off + 256])
            flats[nm] = fl

        # ---------------- Stage E: pipelined chunks -------------------------
        def bcast(pool, k, names, tagp):
            tiles = {}
            for nm in names:
                t = pool.tile([128, CH], F16, tag=tagp + nm[:2])
                fl = flats[nm]
                for h in (0, 1):
                    src = fl[k:k + 1, CH * h:CH * h + CH].unsqueeze(1) \
                        .to_broadcast([1, 64, CH])
                    nc.sync.dma_start(t[64 * h:64 * h + 64, :], src)
                tiles[nm] = t
            return tiles

        def warp1(k):
            cf = bcast(cfF, k, ("amf", "dnf", "dpf"), "F")
            a = k * CH
            o = 1 + a   # xw column offset (halo padding)
            nc.vector.tensor_tensor(xw[:, o:o + CH], x16[:, a:a + CH],
                                    cf["amf"][:], ALU.mult)
            n1 = wrk.tile([128, CH], F16, tag="n1")
            nc.gpsimd.tensor_tensor(n1[:, 0:CH - 1], x16[:, a:a + CH - 1],
                                    cf["dnf"][:, 1:CH], ALU.mult)
            nc.vector.tensor_tensor(xw[:, o + 1:o + CH], xw[:, o + 1:o + CH],
                                    n1[:, 0:CH - 1], ALU.add)
            n2 = wrk.tile([128, CH], F16, tag="n2")
            nc.gpsimd.tensor_tensor(n2[:, 0:CH - 1], x16[:, a + 1:a + CH],
                                    cf["dpf"][:, 0:CH - 1], ALU.mult)
            nc.vector.tensor_tensor(xw[:, o:o + CH - 1], xw[:, o:o + CH - 1],
                                    n2[:, 0:CH - 1], ALU.add)

        def conv(k):
            a = k * CH
            for s in range(CH // SUB):
                pc = psC.tile([128, SUB], F32, tag="pc")
                for j in range(3):
                    nc.tensor.matmul(pc[:], cw_sb[:, 128 * j:128 * j + 128],
                                     xw[:, a + SUB * s + j:a + SUB * s + j + SUB],
                                     start=(j == 0), stop=(j == 2))
                nc.scalar.activation(yc[:, a + SUB * s:a + SUB * s + SUB], pc[:],
                                     AF.Identity, bias=cb_sb[:])

        def warp2(k):
            cf = bcast(cfI, k, ("ami", "dni", "dpi"), "I")
            a = k * CH
            m = wrk.tile([128, CH], F16, tag="m")
            nc.vector.tensor_tensor(m[:], yc[:, a:a + CH], cf["ami"][:],
                                    ALU.mult)
            n1 = wrk.tile([128, CH], F16, tag="n1")
            nc.gpsimd.tensor_tensor(n1[:, 0:CH - 1], yc[:, a:a + CH - 1],
                                    cf["dni"][:, 1:CH], ALU.mult)
            nc.vector.tensor_tensor(m[:, 1:CH], m[:, 1:CH], n1[:, 0:CH - 1],
                                    ALU.add)
            n2 = wrk.tile([128, CH], F16, tag="n2")
            nc.gpsimd.memset(n2[:, CH - 1:CH], 0.0)
            nc.vector.tensor_tensor(n2[:, 0:CH - 1], yc[:, a + 1:a + CH],
                                    cf["dpi"][:, 0:CH - 1], ALU.mult)
            yo = f32s.tile([128, CH], F32, tag="f32s")
            nc.gpsimd.tensor_tensor(yo[:], m[:], n2[:], ALU.add)
            nc.sync.dma_start(y_hc[:, :, a:a + CH], yo[:])

        warp1(NCH - 1)
        warp1(0)
        # cross-half conv halo: half1 left neighbour = half0 last col and v.v.
        nc.sync.dma_start(xw[64:128, 0:1], xw[0:64, H:H + 1])
        nc.sync.dma_start(xw[0:64, H + 1:H + 2], xw[64:128, 1:2])
        warp1(1)
        for k in range(NCH):
            if 2 <= k + 1 < NCH - 1:
                warp1(k + 1)
            conv(k)
            warp2(k)

    nc.compile()
    return nc


def _host_params(est_w, conv_w, conv_b):
    ew = np.zeros((128, 2), np.float16)
    ew[:64, 0] = est_w
    ew[64:, 1] = est_w
    cw = np.zeros((128, 384), np.float16)
    for j in range(3):
        blk = conv_w[:, :, j].T.astype(np.float16)   # (in, out)
        cw[0:64, j * 128:j * 128 + 64] = blk
        cw[64:128, j * 128 + 64:j * 128 + 128] = blk
    cb = np.concatenate([conv_b, conv_b]).astype(np.float32)[:, None]
    return ew, cw, cb


_COMPILED = None


def _get_compiled():
    global _COMPILED
    if _COMPILED is None:
        nc = _build_module()
        nc.m = get_hw_module(nc.m)
        _COMPILED = nc
    return _COMPILED


def kernel(signal, est_w, conv_w, conv_b, _trace=False, _trace_kwargs=None):
    nc = _get_compiled()
    ew, cw, cb = _host_params(np.asarray(est_w, np.float32),
                              np.asarray(conv_w, np.float32),
                              np.asarray(conv_b, np.float32))
    signal = np.ascontiguousarray(np.asarray(signal, np.float32))
    in_maps = [{"x": signal[b], "ew": ew, "cw": cw, "cb": cb}
               for b in range(NCORES)]
    res = bass_utils.run_bass_kernel_spmd(
        nc, in_maps, core_ids=list(range(NCORES)), trace=_trace,
        **(_trace_kwargs or {}))
    out = np.stack([r["y"] for r in res.results], axis=0)
    if _trace:
        return out, res
    return out


# revision 14
# speedup vs baseline: 1.9709x; 1.9709x over previous
"""Trainium2 Bass kernel for gated short-time-warp + Conv1d (nn_GW_Conv1D).

Reference computation (per batch element b, C=64 channels, T=32768):
  g = tanh(einsum('ct,c->t', x, est_w)) * 0.5            # velocity, |g| <= 0.5
  d = flow(g)    per 256-window (scaling & squaring, 4 iters), |d| <= 0.5
  xw = interp1d(x, p + d)   per window                    # forward warp
  y = conv1d(xw, conv_w, conv_b, k=3, SAME)               # channel mixing
  d_inv = flow(-g); out = interp1d(y, p + d_inv)          # inverse warp

Because |d| < 1 always, every interpolation touches only nearest neighbours,
so the warps are 3-term elementwise expressions with relu-split coefficients:
  out = x*am + x[-1]*dn + x[+1]*dp,  dn=relu(-d), dp=relu(d), am=1-dn-dp
with dn/dp zeroed at window edges (replicates jnp.clip at the borders).

Sharding: pure data parallelism, batch b -> core b (8 cores).

Layout: everything stays in "conv layout" (128 partitions = channel + 64*half,
16384 time columns). The warp coefficients are channel-invariant, so they are
computed compactly in window layout (128 windows x 256), flattened to one row
per (half, chunk), and broadcast across the 64 channel partitions per half
with large-descriptor SBUF->SBUF DMAs. This avoids the per-channel 512B
scatter/gather layout conversions entirely.

g is produced by x-stationary matmuls (output = time-on-partitions in PSUM),
moved to window layout via two XBAR DMA transposes + a tiny row permute.

The time axis is processed in 8 chunks of 2048 columns, fully pipelined:
broadcast -> fwd warp (DVE+Pool) -> conv (PE, fp16) -> inv warp -> store.
Chunk 7 is warped first so the cross-half conv halo columns are ready before
conv of chunk 0 runs.
"""
import sys

sys.path.insert(0, "/opt/trn_rl_repo")

import numpy as np
from contextlib import ExitStack

import concourse.bass as bass
import concourse.tile as tile
from concourse import bacc, mybir
from concourse.bass_interp import get_hw_module
from concourse import bass_utils

F32 = mybir.dt.float32
F16 = mybir.dt.float16
AF = mybir.ActivationFunctionType
ALU = mybir.AluOpType

NCORES = 8
C, T, W = 64, 32768, 256
H = T // 2            # 16384 columns per half (stacked-halves conv layout)
CH = 1024             # pipeline chunk width
NCH = H // CH         # 16 chunks
SUB = 512             # conv sub-chunk (one PSUM bank)
FLOW_ITERS = 4


def _flow_iteration(nc, pool, d2):
    """One scaling-and-squaring step on d2 (128, 512) fp16 = [d_fwd | d_inv].
    d2 <- d2 + interp1d(d2, p + d2), per 256-column window."""
    dn = pool.tile([128, 512], F16, tag="fl_dn")
    dp = pool.tile([128, 512], F16, tag="fl_dp")
    nc.scalar.activation(dn[:], d2[:], AF.Relu, scale=-1.0)
    nc.scalar.activation(dp[:], d2[:], AF.Relu)
    # window-edge masking (jnp.clip at borders)
    nc.gpsimd.memset(dn[:, 0:1], 0.0)
    nc.gpsimd.memset(dn[:, 256:257], 0.0)
    nc.gpsimd.memset(dp[:, 255:256], 0.0)
    nc.gpsimd.memset(dp[:, 511:512], 0.0)
    am = pool.tile([128, 512], F16, tag="fl_am")
    nc.vector.tensor_tensor(am[:], dn[:], dp[:], ALU.add)
    nc.vector.tensor_scalar(am[:], am[:], -1.0, 1.0, ALU.mult, ALU.add)
    itp = pool.tile([128, 512], F16, tag="fl_itp")
    tmp = pool.tile([128, 512], F16, tag="fl_tmp")
    nc.vector.tensor_tensor(itp[:], d2[:], am[:], ALU.mult)
    # left-neighbour term (dn masked at window starts -> cross-window leak *0)
    nc.vector.tensor_tensor(tmp[:, 1:512], d2[:, 0:511], dn[:, 1:512], ALU.mult)
    nc.vector.tensor_tensor(itp[:, 1:512], itp[:, 1:512], tmp[:, 1:512], ALU.add)
    # right-neighbour term
    nc.vector.tensor_tensor(tmp[:, 0:511], d2[:, 1:512], dp[:, 0:511], ALU.mult)
    nc.vector.tensor_tensor(itp[:, 0:511], itp[:, 0:511], tmp[:, 0:511], ALU.add)
    nc.vector.tensor_tensor(d2[:], d2[:], itp[:], ALU.add)


def _build_module():
    nc = bacc.Bacc("TRN2", target_bir_lowering=False, debug=False,
                   enable_asserts=False, num_devices=NCORES)
    x = nc.dram_tensor("x", (C, T), F32, kind="ExternalInput").ap()
    ew = nc.dram_tensor("ew", (128, 2), F16, kind="ExternalInput").ap()
    cw = nc.dram_tensor("cw", (128, 384), F16, kind="ExternalInput").ap()
    cb = nc.dram_tensor("cb", (128, 1), F32, kind="ExternalInput").ap()
    y = nc.dram_tensor("y", (C, T), F32, kind="ExternalOutput").ap()

    x_hc = x.rearrange("c (h t) -> h c t", h=2)          # (2, 64, H)
    y_hc = y.rearrange("c (h t) -> h c t", h=2)

    with tile.TileContext(nc) as tc, ExitStack() as ctx:
        sm = ctx.enter_context(tc.tile_pool(name="sm", bufs=1))
        big = ctx.enter_context(tc.tile_pool(name="big", bufs=1))
        f32s = ctx.enter_context(tc.tile_pool(name="f32s", bufs=2))
        cfB = ctx.enter_context(tc.tile_pool(name="cfB", bufs=1))
        wrk = ctx.enter_context(tc.tile_pool(name="wrk", bufs=2))
        psE = ctx.enter_context(tc.tile_pool(name="psE", bufs=1, space="PSUM"))
        psC = ctx.enter_context(tc.tile_pool(name="psC", bufs=4, space="PSUM"))

        ew_sb = sm.tile([128, 2], F16, tag="ew")
        nc.sync.dma_start(ew_sb[:], ew)
        cw_sb = sm.tile([128, 384], F16, tag="cw")
        nc.sync.dma_start(cw_sb[:], cw)
        cb_sb = sm.tile([128, 1], F32, tag="cb")
        nc.sync.dma_start(cb_sb[:], cb)

        x16 = big.tile([128, H], F16)          # signal, conv layout
        xw = big.tile([128, H + 2], F16)       # warped, col0/col H+1 = halo
        yc = big.tile([128, H], F16)           # conv output
        nc.gpsimd.memset(xw[0:64, 0:1], 0.0)          # t=-1 of half0
        nc.gpsimd.memset(xw[64:128, H + 1:H + 2], 0.0)  # t=H of half1

        # ---------------- Stage A: load + cast + einsum g -------------------
        # einsum via x-stationary matmuls: out partition = time-within-128,
        # psum col = 2*subchunk + half.
        # psum column for (subchunk jj, half h) is chosen so that XBAR
        # transposes of the two column halves land DIRECTLY in window layout:
        # col(jj, h) = (jj>>1) + 128*(jj&1) + 64*h.
        g_ps = psE.tile([128, 256], F32, tag="gps")
        g_v = g_ps.rearrange("q (a b) -> q a b", b=64)
        for k in range(NCH):
            a = k * CH
            xf = f32s.tile([128, CH], F32, tag="f32s")
            nc.gpsimd.dma_start(xf[:], x_hc[:, :, a:a + CH],
                                single_packet=True)
            nc.scalar.copy(x16[:, a:a + CH], xf[:])
            for j in range(CH // 128):
                jj = (CH // 128) * k + j
                a0, b0 = 2 * (jj & 1), jj >> 1
                nc.tensor.matmul(g_v[:, a0:a0 + 2, b0:b0 + 1],
                                 x16[:, 128 * jj:128 * jj + 128], ew_sb[:],
                                 start=True, stop=True)

        # ---------------- Stage B: g -> window layout -----------------------
        g16 = sm.tile([128, 256], F16, tag="g16")
        nc.scalar.copy(g16[:], g_ps[:])
        # gw16[f=64h+l, w] = g(h, 256l + w): two direct XBAR transposes
        gw16 = sm.tile([128, 256], F16, tag="gw16")
        nc.sync.dma_start_transpose(gw16[:, 0:128], g16[:, 0:128])
        nc.sync.dma_start_transpose(gw16[:, 128:256], g16[:, 128:256])

        # ---------------- Stage C: flow + coefficients ----------------------
        gth = sm.tile([128, 256], F16, tag="gth")
        nc.scalar.activation(gth[:], gw16[:], AF.Tanh)
        d2 = sm.tile([128, 512], F16, tag="d2")           # [d_fwd | d_inv]
        nc.vector.tensor_scalar_mul(d2[:, 0:256], gth[:], 0.5 / 16.0)
        nc.vector.tensor_scalar_mul(d2[:, 256:512], gth[:], -0.5 / 16.0)
        for _ in range(FLOW_ITERS):
            _flow_iteration(nc, sm, d2)
        dn2 = sm.tile([128, 512], F16, tag="cf_dn")
        dp2 = sm.tile([128, 512], F16, tag="cf_dp")
        nc.scalar.activation(dn2[:], d2[:], AF.Relu, scale=-1.0)
        nc.scalar.activation(dp2[:], d2[:], AF.Relu)
        nc.gpsimd.memset(dn2[:, 0:1], 0.0)
        nc.gpsimd.memset(dn2[:, 256:257], 0.0)
        nc.gpsimd.memset(dp2[:, 255:256], 0.0)
        nc.gpsimd.memset(dp2[:, 511:512], 0.0)

        # ---------------- Stage D: coefficient broadcast --------------------
        # Materialize full-width (128, H) fp16 dn/dp tensors per direction by
        # seeding row h*64 with the flattened window coefficients, then
        # log-doubling down the partition dim (normal strided packets; the
        # last steps are column-split so packets spread across DMA engines).
        # fwd and inv share buffers (pool tag rotation).
        def bcast_dir(off):
            tiles = {}
            calls = []
            for nm, srcv in (("dn", dn2), ("dp", dp2)):
                cb2 = cfB.tile([128, H], F16, tag=nm)
                nc.sync.dma_start(cb2[0:1, :], srcv[0:64, off:off + 256])
                nc.scalar.dma_start(cb2[64:65, :], srcv[64:128, off:off + 256])
                tiles[nm] = cb2
            # breadth-first: one doubling step across all 4 chains at a time
            eng = [nc.sync, nc.scalar]
            ei = 0
            for s, nsplit in ((1, 1), (2, 1), (4, 1), (8, 2), (16, 4), (32, 8)):
                w = H // nsplit
                for nm in ("dn", "dp"):
                    cb2 = tiles[nm]
                    for h in (0, 64):
                        for j in range(nsplit):
                            eng[ei % 2].dma_start(
                                cb2[h + s:h + 2 * s, j * w:(j + 1) * w],
                                cb2[h:h + s, j * w:(j + 1) * w])
                            ei += 1
            return tiles

        # ---------------- Stage E: pipelined chunks -------------------------
        # warp: out = x + dn (.) (xl - x) + dp (.) (xr - x), edge-masked.
        def warp1(k, cf):
            a = k * CH
            o = 1 + a   # xw column offset (halo padding)
            dn_, dp_ = cf["dn"], cf["dp"]
            s1 = wrk.tile([128, CH], F16, tag="s1")
            s2 = wrk.tile([128, CH], F16, tag="s2")
            nc.vector.tensor_tensor(s1[:, 1:CH], x16[:, a:a + CH - 1],
                                    x16[:, a + 1:a + CH], ALU.subtract)
            nc.gpsimd.tensor_tensor(s1[:, 1:CH], s1[:, 1:CH],
                                    dn_[:, a + 1:a + CH], ALU.mult)
            nc.vector.tensor_tensor(s2[:, 0:CH - 1], x16[:, a + 1:a + CH],
                                    x16[:, a:a + CH - 1], ALU.subtract)
            nc.gpsimd.tensor_tensor(s2[:, 0:CH - 1], s2[:, 0:CH - 1],
                                    dp_[:, a:a + CH - 1], ALU.mult)
            nc.vector.tensor_tensor(xw[:, o + 1:o + CH], x16[:, a + 1:a + CH],
                                    s1[:, 1:CH], ALU.add)
            nc.scalar.copy(xw[:, o:o + 1], x16[:, a:a + 1])
            nc.vector.tensor_tensor(xw[:, o:o + CH - 1], xw[:, o:o + CH - 1],
                                    s2[:, 0:CH - 1], ALU.add)

        def conv(k):
            a = k * CH
            for s in range(CH // SUB):
                pc = psC.tile([128, SUB], F32, tag="pc")
                for j in range(3):
                    nc.tensor.matmul(pc[:], cw_sb[:, 128 * j:128 * j + 128],
                                     xw[:, a + SUB * s + j:a + SUB * s + j + SUB],
                                     start=(j == 0), stop=(j == 2))
                nc.scalar.activation(yc[:, a + SUB * s:a + SUB * s + SUB], pc[:],
                                     AF.Identity, bias=cb_sb[:])

        def warp2(k, cf):
            a = k * CH
            dn_, dp_ = cf["dn"], cf["dp"]
            s1 = wrk.tile([128, CH], F16, tag="s1")
            s2 = wrk.tile([128, CH], F16, tag="s2")
            nc.gpsimd.memset(s1[:, 0:1], 0.0)
            nc.gpsimd.memset(s2[:, CH - 1:CH], 0.0)
            nc.vector.tensor_tensor(s1[:, 1:CH], yc[:, a:a + CH - 1],
                                    yc[:, a + 1:a + CH], ALU.subtract)
            nc.vector.tensor_tensor(s1[:, 1:CH], s1[:, 1:CH],
                                    dn_[:, a + 1:a + CH], ALU.mult)
            nc.vector.tensor_tensor(s2[:, 0:CH - 1], yc[:, a + 1:a + CH],
                                    yc[:, a:a + CH - 1], ALU.subtract)
            nc.gpsimd.tensor_tensor(s2[:, 0:CH - 1], s2[:, 0:CH - 1],
                                    dp_[:, a:a + CH - 1], ALU.mult)
            yo = f32s.tile([128, CH], F32, tag="f32s")
            nc.gpsimd.tensor_tensor(yo[:], yc[:, a:a + CH], s1[:], ALU.add)
            nc.gpsimd.tensor_tensor(yo[:], yo[:], s2[:], ALU.add)
            nc.gpsimd.dma_start(y_hc[:, :, a:a + CH], yo[:],
                                single_packet=True)

        cfF = bcast_dir(0)
        warp1(NCH - 1, cfF)
        warp1(0, cfF)
        # cross-half conv halo: half1 left neighbour = half0 last col and v.v.
        nc.sync.dma_start(xw[64:128, 0:1], xw[0:64, H:H + 1])
        nc.sync.dma_start(xw[0:64, H + 1:H + 2], xw[64:128, 1:2])
        warp1(1, cfF)
        for k in range(NCH):
            if 2 <= k + 1 < NCH - 1:
                warp1(k + 1, cfF)
            conv(k)
        cfI = bcast_dir(256)
        for k in range(NCH):
            warp2(k, cfI)

    nc.compile()
    return nc


def _host_params(est_w, conv_w, conv_b):
    ew = np.zeros((128, 2), np.float16)
    ew[:64, 0] = est_w
    ew[64:, 1] = est_w
    cw = np.zeros((128, 384), np.float16)
    for j in range(3):
        blk = conv_w[:, :, j].T.astype(np.float16)   # (in, out)
        cw[0:64, j * 128:j * 128 + 64] = blk
        cw[64:128, j * 128 + 64:j * 128 + 128] = blk
    cb = np.concatenate([conv_b, conv_b]).astype(np.float32)[:, None]
    return ew, cw, cb


_COMPILED = None


def _get_compiled():
    global _COMPILED
    if _COMPILED is None:
        nc = _build_module()
        nc.m = get_hw_module(nc.m)
        _COMPILED = nc
    return _COMPILED


def kernel(signal, est_w, conv_w, conv_b, _trace=False, _trace_kwargs=None):
    nc = _get_compiled()
    ew, cw, cb = _host_params(np.asarray(est_w, np.float32),
                              np.asarray(conv_w, np.float32),
                              np.asarray(conv_b, np.float32))
    signal = np.ascontiguousarray(np.asarray(signal, np.float32))
    in_maps = [{"x": signal[b], "ew": ew, "cw": cw, "cb": cb}
               for b in range(NCORES)]
    res = bass_utils.run_bass_kernel_spmd(
        nc, in_maps, core_ids=list(range(NCORES)), trace=_trace,
        **(_trace_kwargs or {}))
    out = np.stack([r["y"] for r in res.results], axis=0)
    if _trace:
        return out, res
    return out
